# revision 4
# baseline (speedup 1.0000x reference)
"""Trainium2 Bass kernel for 1D correlation layer (FlowNet-style).

Problem (hardcoded):
  x_1, x_2: [B=8, C=256, H=96, W=320] fp32
  out[b, d, h, w] = sum_c x_1[b,c,h,w] * x_2p[b,c,h,w+d],  d in [0, 41)
  where x_2p is x_2 zero-padded by 20 on each side of W.

Sharding: data-parallel over batch B across the 8 NeuronCores (one sample
per core); correlation has no cross-batch interaction.

v3: this problem is DMA-bus-bound (in 31.5MB bf16 + staged out share one
~340GB/s bus), so everything optimizes bytes:
  - inputs host-converted to bf16 and host-permuted to [hg, c, hh, w] so
    each input DMA reads one fully contiguous 1.31MB DRAM block;
  - bf16 matmuls stream only each 64-row w-tile's band (84/104 cols);
    contraction over C = 2 accumulating K=128 matmuls; 4 h-planes of a
    w-tile pair share one PSUM bank [128, 4, 104] fp32;
  - PSUM->SBUF staging casts to bf16 and trims the band per 32-row
    sub-block (rows [0,32) keep v in [0,72), rows [32,64) keep v in
    [32,104)) via two-way partition-strided copy APs -> 4.42MB staged;
  - the final fine shear (out[d, w] = G[w, w+d]) is a strided-view gather
    on the host during unshard - all MACs and all data streaming happen on
    device; the host only reindexes device-produced values and zeroes the
    fixed out-of-range triangles at the W edges.
"""

import numpy as np

B, C, H, W = 8, 256, 96, 320
MAX_DISP = 20
D = 2 * MAX_DISP + 1  # 41
NCORES = 8

HG = 8                  # h-planes per group
NHG = H // HG           # 12 groups
JG = 4                  # h-planes packed per PSUM tile
NJG = HG // JG          # 2
MT = 64                 # w-tile size
NWT = W // MT           # 5 w-tiles
VW = MT + 2 * MAX_DISP  # 104 band columns per 64-row block
SB = 32                 # staging sub-block rows
SVW = SB + 2 * MAX_DISP  # 72 staged band columns per 32-row sub-block
NPAIR = (NWT + 1) // 2  # 3 stage pairs (last one half-filled)
# per w-tile band: (a, ncols, vlo); moving x2 cols [a, a+ncols) land at
# PSUM v-positions [vlo, vlo+ncols); v = u - (64*wb - 20)
WTILES = []
for _wb in range(NWT):
    _lo = max(0, 64 * _wb - MAX_DISP)
    _hi = min(W, 64 * _wb + MT + MAX_DISP)
    WTILES.append((_lo, _hi - _lo, _lo - (64 * _wb - MAX_DISP)))

_nc_cache = {}


def _build(reps=1, ablate="full"):
    # ablate: "full" | "in" (input DMA only) | "in+mm" (no copies/out-DMA)
    import concourse.bacc as bacc
    import concourse.tile as tile
    import concourse.mybir as mybir
    import contextlib

    nc = bacc.Bacc(
        "TRN2",
        target_bir_lowering=False,
        debug=False,
        enable_asserts=False,
        num_devices=NCORES,
    )
    f32 = mybir.dt.float32
    bf16 = mybir.dt.bfloat16

    x1 = nc.dram_tensor("x_1", (NHG, C, HG, W), bf16, kind="ExternalInput").ap()
    x2 = nc.dram_tensor("x_2", (NHG, C, HG, W), bf16, kind="ExternalInput").ap()
    # scr[p, hg, r, hh, v]: r packs four 32-row sub-blocks of the w-tile
    # pair (2p + r//64, sub r%64//32); within a sub-block, row q = r%32 is
    # w = 64*wb + 32*sub + q and v = q + d (d in [0, 41)).
    scr = nc.dram_tensor(
        "out_scr", (NPAIR, NHG, 2 * MT, HG, SVW), bf16, kind="ExternalOutput"
    ).ap()

    with tile.TileContext(nc) as tc:
        with tc.tile_pool(name="xin", bufs=2) as xpool, \
             tc.tile_pool(name="stg", bufs=2) as spool, \
             tc.tile_pool(name="ps", bufs=8, space="PSUM") as ppool:
            loop_ctx = tc.For_i(0, reps, 1) if reps > 1 else contextlib.nullcontext()
            with loop_ctx:
                for hg in range(NHG):
                    x1t = []
                    x2t = []
                    for ck in range(2):
                        t1 = xpool.tile(
                            [128, HG * W], bf16, name=f"x1_{hg}_{ck}", tag=f"x1c{ck}",
                        )
                        nc.sync.dma_start(
                            out=t1, in_=x1[hg, ck * 128:(ck + 1) * 128],
                        )
                        x1t.append(t1)
                        t2 = xpool.tile(
                            [128, HG * W], bf16, name=f"x2_{hg}_{ck}", tag=f"x2c{ck}",
                        )
                        nc.sync.dma_start(
                            out=t2, in_=x2[hg, ck * 128:(ck + 1) * 128],
                        )
                        x2t.append(t2)

                    stages = []
                    for pi in range(NPAIR):
                        st = spool.tile(
                            [2 * MT, HG, SVW], bf16,
                            name=f"st_{hg}_{pi}", tag=f"st{pi}",
                        )
                        stages.append(st)

                    if ablate != "in":
                        for jg in range(NJG):
                            for pi in range(NPAIR):
                                nwb = 2 if 2 * pi + 1 < NWT else 1
                                ps = ppool.tile(
                                    [128, JG, VW], f32,
                                    name=f"ps_{hg}_{jg}_{pi}", tag="ps",
                                )
                                for j in range(JG):
                                    hh = jg * JG + j
                                    for wi in range(nwb):
                                        wb = 2 * pi + wi
                                        a, ncols, vlo = WTILES[wb]
                                        r0 = MT * wi
                                        nc.tensor.matmul(
                                            ps[r0:r0 + MT, j, vlo:vlo + ncols],
                                            x1t[0][:, hh * W + 64 * wb:
                                                   hh * W + 64 * wb + MT],
                                            x2t[0][:, hh * W + a:
                                                   hh * W + a + ncols],
                                            start=True, stop=False,
                                        )
                                        nc.tensor.matmul(
                                            ps[r0:r0 + MT, j, vlo:vlo + ncols],
                                            x1t[1][:, hh * W + 64 * wb:
                                                   hh * W + 64 * wb + MT],
                                            x2t[1][:, hh * W + a:
                                                   hh * W + a + ncols],
                                            start=False, stop=True,
                                        )
                                if ablate == "in+mm":
                                    continue
                                # stage sub-blocks: rows [0,32) of a w-tile
                                # keep v in [0,72); rows [32,64) keep
                                # [32,104) (band trim, bf16 cast)
                                hs = slice(jg * JG, (jg + 1) * JG)
                                for wi in range(nwb):
                                    for sub in range(2):
                                        r0 = MT * wi + SB * sub
                                        dst = stages[pi][r0:r0 + SB, hs, :]
                                        src = ps[r0:r0 + SB, :,
                                                 sub * SB:sub * SB + SVW]
                                        if (jg + wi + sub) % 2 == 0:
                                            nc.vector.tensor_copy(dst, src)
                                        else:
                                            nc.scalar.copy(dst, src)

                    if ablate == "full":
                        for pi in range(NPAIR):
                            rows = 2 * MT if 2 * pi + 1 < NWT else MT
                            nc.sync.dma_start(
                                out=scr[pi, hg, 0:rows],
                                in_=stages[pi][0:rows],
                            )

    nc.compile()
    return nc


def _get_nc(reps=1, ablate="full"):
    key = (reps, ablate)
    if key not in _nc_cache:
        _nc_cache[key] = _build(reps, ablate)
    return _nc_cache[key]


def _unshear(scr_np, out):
    """scr[p, hg, r, hh, v] -> out[d, h, w]; 32-row sub-block j32 = 2*(r//64)
    + (r%64)//32 of pair p, q = r%32, w = 64*(2p + r//64) + 32*((r%64)//32)
    + q, h = hg*8 + hh, v = q + d."""
    out_r = out.reshape(D, NHG, HG, 2 * NWT, SB)
    for w32 in range(2 * NWT):
        wb = w32 // 2
        r0 = MT * (wb % 2) + SB * (w32 % 2)
        block = scr_np[wb // 2, :, r0:r0 + SB]
        bs = block.strides  # [NHG, SB, HG, SVW]
        v = np.lib.stride_tricks.as_strided(
            block,
            shape=(NHG, SB, HG, D),
            strides=(bs[0], bs[1] + bs[3], bs[2], bs[3]),
        )
        # v[hg, q, hh, d] -> out[d, hg, hh, w32, q]
        out_r[:, :, :, w32, :] = v.transpose(3, 0, 2, 1)
    # zero the out-of-range shift positions (reference zero-pads x_2 in W)
    for w in range(MAX_DISP):
        out[:MAX_DISP - w, :, w] = 0.0
    for w in range(W - MAX_DISP, W):
        out[(W + MAX_DISP - 1) - w + 1:, :, w] = 0.0
    return out


def kernel(x_1, x_2):
    import ml_dtypes
    from concourse.bass_utils import run_bass_kernel_spmd

    x_1 = np.asarray(x_1)
    x_2 = np.asarray(x_2)
    assert x_1.shape == (B, C, H, W) and x_2.shape == (B, C, H, W)
    # bf16 + [hg, c, hh, w] layout (contiguous per input DMA)
    xb1 = np.ascontiguousarray(
        x_1.astype(ml_dtypes.bfloat16)
        .reshape(B, C, NHG, HG, W).transpose(0, 2, 1, 3, 4))
    xb2 = np.ascontiguousarray(
        x_2.astype(ml_dtypes.bfloat16)
        .reshape(B, C, NHG, HG, W).transpose(0, 2, 1, 3, 4))

    nc = _get_nc(1)
    in_maps = [{"x_1": xb1[b], "x_2": xb2[b]} for b in range(NCORES)]
    res = run_bass_kernel_spmd(nc, in_maps, core_ids=list(range(NCORES)))
    out = np.empty((B, D, H, W), np.float32)
    for b in range(NCORES):
        _unshear(res.results[b]["out_scr"], out[b])
    return out


# revision 6
# speedup vs baseline: 1.1022x; 1.1022x over previous
"""Trainium2 Bass kernel for 1D correlation layer (FlowNet-style).

Problem (hardcoded):
  x_1, x_2: [B=8, C=256, H=96, W=320] fp32
  out[b, d, h, w] = sum_c x_1[b,c,h,w] * x_2p[b,c,h,w+d],  d in [0, 41)
  where x_2p is x_2 zero-padded by 20 on each side of W.

Sharding: data-parallel over batch B across the 8 NeuronCores (one sample
per core); correlation has no cross-batch interaction.

v3: this problem is DMA-bus-bound (in 31.5MB bf16 + staged out share one
~340GB/s bus), so everything optimizes bytes:
  - inputs host-converted to bf16 and host-permuted to [hg, c, hh, w] so
    each input DMA reads one fully contiguous 1.31MB DRAM block;
  - bf16 matmuls stream only each 64-row w-tile's band (84/104 cols);
    contraction over C = 2 accumulating K=128 matmuls; 4 h-planes of a
    w-tile pair share one PSUM bank [128, 4, 104] fp32;
  - PSUM->SBUF staging casts to bf16 and trims the band per 32-row
    sub-block (rows [0,32) keep v in [0,72), rows [32,64) keep v in
    [32,104)) via two-way partition-strided copy APs -> 4.42MB staged;
  - the final fine shear (out[d, w] = G[w, w+d]) is a strided-view gather
    on the host during unshard - all MACs and all data streaming happen on
    device; the host only reindexes device-produced values and zeroes the
    fixed out-of-range triangles at the W edges.
"""

import numpy as np

B, C, H, W = 8, 256, 96, 320
MAX_DISP = 20
D = 2 * MAX_DISP + 1  # 41
NCORES = 8

HG = 8                  # h-planes per group
NHG = H // HG           # 12 groups
JG = 4                  # h-planes packed per PSUM tile
NJG = HG // JG          # 2
MT = 64                 # w-tile size
NWT = W // MT           # 5 w-tiles
VW = MT + 2 * MAX_DISP  # 104 band columns per 64-row block
SB = 32                 # staging sub-block rows
SVW = SB + 2 * MAX_DISP  # 72 staged band columns per 32-row sub-block
NPAIR = (NWT + 1) // 2  # 3 stage pairs (last one half-filled)
# per w-tile band: (a, ncols, vlo); moving x2 cols [a, a+ncols) land at
# PSUM v-positions [vlo, vlo+ncols); v = u - (64*wb - 20)
WTILES = []
for _wb in range(NWT):
    _lo = max(0, 64 * _wb - MAX_DISP)
    _hi = min(W, 64 * _wb + MT + MAX_DISP)
    WTILES.append((_lo, _hi - _lo, _lo - (64 * _wb - MAX_DISP)))

_nc_cache = {}


def _build(reps=1, ablate="full"):
    # ablate: "full" | "in" (input DMA only) | "in+mm" (no copies/out-DMA)
    import concourse.bacc as bacc
    import concourse.tile as tile
    import concourse.mybir as mybir
    import contextlib

    nc = bacc.Bacc(
        "TRN2",
        target_bir_lowering=False,
        debug=False,
        enable_asserts=False,
        num_devices=NCORES,
    )
    f32 = mybir.dt.float32
    bf16 = mybir.dt.bfloat16

    # [C, H, W] keeps each input DMA's per-partition chunks DRAM-strided:
    # 128 x 5120B descriptors split evenly over the 16 DMA engines (a fully
    # contiguous layout fuses into ~10 64KB descriptors and loses ~35% BW)
    x1 = nc.dram_tensor("x_1", (C, H, W), bf16, kind="ExternalInput").ap()
    x2 = nc.dram_tensor("x_2", (C, H, W), bf16, kind="ExternalInput").ap()
    # scr[p, hg, r, hh, v]: r packs four 32-row sub-blocks of the w-tile
    # pair (2p + r//64, sub r%64//32); within a sub-block, row q = r%32 is
    # w = 64*wb + 32*sub + q and v = q + d (d in [0, 41)).
    scr = nc.dram_tensor(
        "out_scr", (NPAIR, NHG, 2 * MT, HG, SVW), bf16, kind="ExternalOutput"
    ).ap()

    with tile.TileContext(nc) as tc:
        with tc.tile_pool(name="xin", bufs=2) as xpool, \
             tc.tile_pool(name="stg", bufs=2) as spool, \
             tc.tile_pool(name="ps", bufs=8, space="PSUM") as ppool:
            loop_ctx = tc.For_i(0, reps, 1) if reps > 1 else contextlib.nullcontext()
            with loop_ctx:
                for hg in range(NHG):
                    x1t = []
                    x2t = []
                    for ck in range(2):
                        t1 = xpool.tile(
                            [128, HG * W], bf16, name=f"x1_{hg}_{ck}", tag=f"x1c{ck}",
                        )
                        nc.sync.dma_start(
                            out=t1,
                            in_=x1[ck * 128:(ck + 1) * 128,
                                   hg * HG:(hg + 1) * HG, :],
                        )
                        x1t.append(t1)
                        t2 = xpool.tile(
                            [128, HG * W], bf16, name=f"x2_{hg}_{ck}", tag=f"x2c{ck}",
                        )
                        nc.sync.dma_start(
                            out=t2,
                            in_=x2[ck * 128:(ck + 1) * 128,
                                   hg * HG:(hg + 1) * HG, :],
                        )
                        x2t.append(t2)

                    stages = []
                    for pi in range(NPAIR):
                        st = spool.tile(
                            [2 * MT, HG, SVW], bf16,
                            name=f"st_{hg}_{pi}", tag=f"st{pi}",
                        )
                        stages.append(st)

                    if ablate != "in":
                        for jg in range(NJG):
                            for pi in range(NPAIR):
                                nwb = 2 if 2 * pi + 1 < NWT else 1
                                ps = ppool.tile(
                                    [128, JG, VW], f32,
                                    name=f"ps_{hg}_{jg}_{pi}", tag="ps",
                                )
                                for j in range(JG):
                                    hh = jg * JG + j
                                    for wi in range(nwb):
                                        wb = 2 * pi + wi
                                        a, ncols, vlo = WTILES[wb]
                                        r0 = MT * wi
                                        nc.tensor.matmul(
                                            ps[r0:r0 + MT, j, vlo:vlo + ncols],
                                            x1t[0][:, hh * W + 64 * wb:
                                                   hh * W + 64 * wb + MT],
                                            x2t[0][:, hh * W + a:
                                                   hh * W + a + ncols],
                                            start=True, stop=False,
                                        )
                                        nc.tensor.matmul(
                                            ps[r0:r0 + MT, j, vlo:vlo + ncols],
                                            x1t[1][:, hh * W + 64 * wb:
                                                   hh * W + 64 * wb + MT],
                                            x2t[1][:, hh * W + a:
                                                   hh * W + a + ncols],
                                            start=False, stop=True,
                                        )
                                if ablate == "in+mm":
                                    continue
                                # stage sub-blocks: rows [0,32) of a w-tile
                                # keep v in [0,72); rows [32,64) keep
                                # [32,104) (band trim, bf16 cast)
                                hs = slice(jg * JG, (jg + 1) * JG)
                                for wi in range(nwb):
                                    for sub in range(2):
                                        r0 = MT * wi + SB * sub
                                        dst = stages[pi][r0:r0 + SB, hs, :]
                                        src = ps[r0:r0 + SB, :,
                                                 sub * SB:sub * SB + SVW]
                                        if (jg + wi + sub) % 2 == 0:
                                            nc.vector.tensor_copy(dst, src)
                                        else:
                                            nc.scalar.copy(dst, src)

                    if ablate == "full":
                        for pi in range(NPAIR):
                            rows = 2 * MT if 2 * pi + 1 < NWT else MT
                            nc.sync.dma_start(
                                out=scr[pi, hg, 0:rows],
                                in_=stages[pi][0:rows],
                            )

    nc.compile()
    return nc


def _get_nc(reps=1, ablate="full"):
    key = (reps, ablate)
    if key not in _nc_cache:
        _nc_cache[key] = _build(reps, ablate)
    return _nc_cache[key]


def _unshear(scr_np, out):
    """scr[p, hg, r, hh, v] -> out[d, h, w]; 32-row sub-block j32 = 2*(r//64)
    + (r%64)//32 of pair p, q = r%32, w = 64*(2p + r//64) + 32*((r%64)//32)
    + q, h = hg*8 + hh, v = q + d."""
    out_r = out.reshape(D, NHG, HG, 2 * NWT, SB)
    for w32 in range(2 * NWT):
        wb = w32 // 2
        r0 = MT * (wb % 2) + SB * (w32 % 2)
        block = scr_np[wb // 2, :, r0:r0 + SB]
        bs = block.strides  # [NHG, SB, HG, SVW]
        v = np.lib.stride_tricks.as_strided(
            block,
            shape=(NHG, SB, HG, D),
            strides=(bs[0], bs[1] + bs[3], bs[2], bs[3]),
        )
        # v[hg, q, hh, d] -> out[d, hg, hh, w32, q]
        out_r[:, :, :, w32, :] = v.transpose(3, 0, 2, 1)
    # zero the out-of-range shift positions (reference zero-pads x_2 in W)
    for w in range(MAX_DISP):
        out[:MAX_DISP - w, :, w] = 0.0
    for w in range(W - MAX_DISP, W):
        out[(W + MAX_DISP - 1) - w + 1:, :, w] = 0.0
    return out


def kernel(x_1, x_2):
    import ml_dtypes
    from concourse.bass_utils import run_bass_kernel_spmd

    x_1 = np.asarray(x_1)
    x_2 = np.asarray(x_2)
    assert x_1.shape == (B, C, H, W) and x_2.shape == (B, C, H, W)
    xb1 = x_1.astype(ml_dtypes.bfloat16)
    xb2 = x_2.astype(ml_dtypes.bfloat16)

    nc = _get_nc(1)
    in_maps = [
        {"x_1": np.ascontiguousarray(xb1[b]), "x_2": np.ascontiguousarray(xb2[b])}
        for b in range(NCORES)
    ]
    res = run_bass_kernel_spmd(nc, in_maps, core_ids=list(range(NCORES)))
    out = np.empty((B, D, H, W), np.float32)
    for b in range(NCORES):
        _unshear(res.results[b]["out_scr"], out[b])
    return out
